# revision 16
# baseline (speedup 1.0000x reference)
"""Trainium2 Bass kernel for NodeAttAggregator (masked self-attention aggregation).

Math (per reference):
    q = x @ Wq ; k = x @ Wk                       [E,128]
    s = (q @ k.T) / sqrt(128)                     [E,E]
    p = softmax(s masked where mask==0 -> -inf)   row-wise
    out = p @ k                                   [E,128]

Sharding: row-parallel over 8 cores. Core c owns query rows [c*1024,(c+1)*1024).
Each core receives the full x (for k), its local x slice (for q), and its local
mask rows; it computes its 1024 output rows. Host concatenates.

On-core algorithm (per core, R=1024 local rows):
  setup: transpose x on PE (identity matmuls) -> xT; k = x@Wk; kT = Wk.T@xT;
         qT = Wq.T@xT_local; k_aug = bf16(k) with a ones column (denominator).
  main loop over 64 key chunks j (128 keys each):
     ST_psum[kpos,qpos] = kT_chunk.T-style matmul (scores TRANSPOSED, so the
         exp output is directly P.T, which the PV matmul needs as weights)
       + 32768*mask.T via matmul(lhsT=mask_tile, rhs=1024*I) which transposes
         the mask on the PE for free while accumulating into PSUM
     PT = exp(scale*ST + bias) on ACT (PSUM->SBUF, bf16)   [softmax w/o rowmax:
         constant shift C=8; masked lanes underflow to ~0]
     PV: out_psum[m,129] += PT_chunk.T @ k_aug_chunk  (ones col = denominator)
  final: out = out_psum[:, :128] * (1/out_psum[:,128]) ; DMA out.

Unnormalized-softmax trick: out_row = sum_j p_j k_j / sum_j p_j, so no
per-row max/denominator pass is needed; denominator ~ e^{smax-C} stays in
fp32/bf16 range for this data (scores/sqrt(d) in [-19.6, 18.5], checked).
"""

import sys

for _p in ("/opt/trn_rl_repo",):
    if _p not in sys.path:
        sys.path.insert(0, _p)

from contextlib import ExitStack

import numpy as np

import concourse.bacc as bacc
import concourse.bass as bass
import concourse.tile as tile
from concourse import masks, mybir
from concourse import bass_utils

E = 8192
IN_DIM = 256
OUT_DIM = 128
NCORES = 8
R = E // NCORES            # 1024 local query rows per core
P = 128                    # partitions
NJ = E // P                # 64 key chunks
NT = R // P                # 8 local row tiles
DH = IN_DIM // P           # 2 halves of the contraction dim
SCALE = 1.0 / float(np.sqrt(np.float32(OUT_DIM)))
CMASK = 1024.0             # additive mask constant (applied via identity matmul)
CSHIFT = 8.0               # constant softmax shift (instead of row max)
EXP_BIAS = -CMASK * SCALE - CSHIFT

f32 = mybir.dt.float32
bf16 = mybir.dt.bfloat16
i32 = mybir.dt.int32


def _body(ctx: ExitStack, tc: tile.TileContext, out_d, x_d, xl_d, wq_d, wk_d, mask_d,
          dbg=None):
    nc = tc.nc

    # DRAM views
    x_r = x_d.rearrange("(c p) d -> p c d", p=P)          # [128, 64, 256]
    xl_r = xl_d.rearrange("(c p) d -> p c d", p=P)        # [128, 8, 256]
    wq_r = wq_d.rearrange("(h p) d -> p h d", p=P)        # [128, 2, 128]
    wk_r = wk_d.rearrange("(h p) d -> p h d", p=P)
    mask_r = mask_d.rearrange("(t p) (j f) -> p t j f", p=P, f=P)  # [128,8,64,128]
    out_r = out_d.rearrange("(t p) d -> p t d", p=P)      # [128, 8, 128]

    # --- persistent tiles -------------------------------------------------
    persist = ctx.enter_context(tc.tile_pool(name="persist", bufs=1))
    ident_f = persist.tile([P, P], f32, tag="ident_f")
    ident_c = persist.tile([P, P], bf16, tag="ident_c")     # CMASK * I
    bias_sb = persist.tile([P, 1], f32, tag="bias")
    nc.gpsimd.memset(bias_sb[:], EXP_BIAS)
    kT = persist.tile([P, E], f32, tag="kT")                # k transposed [d, kpos]
    qT = persist.tile([P, R], f32, tag="qT")                # q transposed [d, m]
    k_aug = persist.tile([P, NJ, 132], bf16, tag="k_aug")   # k chunks + ones col
    wq_sb = persist.tile([P, DH, P], f32, tag="wq")
    wk_sb = persist.tile([P, DH, P], f32, tag="wk")

    masks.make_identity(nc, ident_f[:])
    nc.gpsimd.memset(ident_c[:], 0.0)
    nc.gpsimd.affine_select(
        out=ident_c[:], in_=ident_c[:],
        compare_op=mybir.AluOpType.not_equal,
        fill=CMASK, base=0, pattern=[[-1, P]], channel_multiplier=1,
    )
    nc.sync.dma_start(wq_sb[:], wq_r)
    nc.sync.dma_start(wk_sb[:], wk_r)
    nc.gpsimd.memset(k_aug[:, :, 128:129], 1.0)

    # --- setup phase: xT, k, kT, qT --------------------------------------
    with tc.tile_pool(name="setup_sb", bufs=2) as ssb, \
         tc.tile_pool(name="setup_xT", bufs=1) as sxT, \
         tc.tile_pool(name="ps_a", bufs=3, space="PSUM") as psa, \
         tc.tile_pool(name="ps_b", bufs=2, space="PSUM") as psb:

        xT = sxT.tile([P, DH, E], f32, tag="xT")        # x transposed (full)
        xTl = sxT.tile([P, DH, R], f32, tag="xTl")      # x_local transposed

        # transpose full x: 64 chunks x 2 halves
        for cc in range(8):
            x_sb = ssb.tile([P, 8, IN_DIM], f32, tag="x_sb")
            nc.sync.dma_start(x_sb[:], x_r[:, cc * 8:(cc + 1) * 8, :])
            for rc in range(8):
                col = cc * 8 + rc
                for h in range(DH):
                    pt = psa.tile([P, P], f32, tag="tr")
                    nc.tensor.transpose(
                        pt[:], x_sb[:, rc, h * P:(h + 1) * P], ident_f[:])
                    nc.any.tensor_copy(xT[:, h, col * P:(col + 1) * P], pt[:])
        # transpose x_local
        for rc in range(NT):
            xl_sb = ssb.tile([P, IN_DIM], f32, tag="xl_sb")
            nc.sync.dma_start(xl_sb[:], xl_r[:, rc, :])
            for h in range(DH):
                pt = psa.tile([P, P], f32, tag="tr")
                nc.tensor.transpose(pt[:], xl_sb[:, h * P:(h + 1) * P], ident_f[:])
                nc.any.tensor_copy(xTl[:, h, rc * P:(rc + 1) * P], pt[:])

        # k (natural) -> k_aug bf16: k_chunk = x_chunk @ Wk
        for col in range(NJ):
            pk = psa.tile([P, P], f32, tag="tr")
            for h in range(DH):
                nc.tensor.matmul(
                    pk[:], xT[:, h, col * P:(col + 1) * P], wk_sb[:, h, :],
                    start=(h == 0), stop=(h == DH - 1))
            nc.any.tensor_copy(k_aug[:, col, 0:128], pk[:])

        # kT = Wk.T @ xT   [d, kpos]
        for n in range(E // 512):
            pkt = psb.tile([P, 512], f32, tag="ktproj")
            for h in range(DH):
                nc.tensor.matmul(
                    pkt[:], wk_sb[:, h, :], xT[:, h, n * 512:(n + 1) * 512],
                    start=(h == 0), stop=(h == DH - 1))
            nc.any.tensor_copy(kT[:, n * 512:(n + 1) * 512], pkt[:])

        # qT = Wq.T @ xT_local   [d, m]
        for n in range(R // 512):
            pqt = psb.tile([P, 512], f32, tag="ktproj")
            for h in range(DH):
                nc.tensor.matmul(
                    pqt[:], wq_sb[:, h, :], xTl[:, h, n * 512:(n + 1) * 512],
                    start=(h == 0), stop=(h == DH - 1))
            nc.any.tensor_copy(qT[:, n * 512:(n + 1) * 512], pqt[:])

    # --- main loop --------------------------------------------------------
    mi_pool = ctx.enter_context(tc.tile_pool(name="mask_i32", bufs=3))
    mb_pool = ctx.enter_context(tc.tile_pool(name="mask_bf16", bufs=2))
    pt_pool = ctx.enter_context(tc.tile_pool(name="pt", bufs=3))
    st_pool = ctx.enter_context(tc.tile_pool(name="st_psum", bufs=2, space="PSUM"))
    pv_pool = ctx.enter_context(tc.tile_pool(name="pv_psum", bufs=1, space="PSUM"))

    pv = pv_pool.tile([P, 1536], f32)   # 3 banks; 8 slots of 129 (+ ones col)

    def pv_col(mt):
        g, s = divmod(mt, 3)
        return g * 512 + s * 129

    for j in range(NJ):
        mi = mi_pool.tile([P, NT, P], i32)
        nc.sync.dma_start(mi[:], mask_r[:, :, j, :])
        mb = mb_pool.tile([P, NT, P], bf16)
        nc.vector.tensor_copy(mb[:], mi[:])

        st = st_pool.tile([P, R], f32)   # [kpos 128, qpos 1024] = 2 banks
        # scores (transposed): st = kT_chunk.T-weights @ qT
        for n in range(R // 512):
            nc.tensor.matmul(
                st[:, n * 512:(n + 1) * 512],
                kT[:, j * P:(j + 1) * P], qT[:, n * 512:(n + 1) * 512],
                start=True, stop=False, skip_group_check=True)
        # + CMASK * mask.T  (PE transposes the mask while accumulating)
        for t in range(NT):
            nc.tensor.matmul(
                st[:, t * P:(t + 1) * P],
                mb[:, t, :], ident_c[:],
                start=False, stop=True, skip_group_check=True)

        ptile = pt_pool.tile([P, R], bf16)
        nc.scalar.activation(
            ptile[:], st[:], mybir.ActivationFunctionType.Exp,
            bias=bias_sb[:], scale=SCALE)
        if dbg is not None and j == 0:
            nc.sync.dma_start(dbg["pt0"], ptile[:])
            stc = pt_pool.tile([P, R], f32, tag="dbg_st")
            nc.vector.tensor_copy(stc[:], st[:])
            nc.sync.dma_start(dbg["st0"], stc[:])

        # start=True clears the whole PSUM bank -> only the FIRST slot of
        # each bank may set it (at j==0), or later slots wipe earlier ones.
        for mt in range(NT):
            g, s = divmod(mt, 3)
            last_in_bank = (mt == NT - 1) or (s == 2)
            c0 = pv_col(mt)
            nc.tensor.matmul(
                pv[:, c0:c0 + 129],
                ptile[:, mt * P:(mt + 1) * P], k_aug[:, j, 0:129],
                start=(j == 0 and s == 0),
                stop=(j == NJ - 1 and last_in_bank),
                skip_group_check=True)

    if dbg is not None:
        nc.sync.dma_start(dbg["qT"], qT[:])
        nc.sync.dma_start(dbg["kT"], kT[:])
        nc.sync.dma_start(dbg["kaug"], k_aug[:, :, 0:129])
        pvc = pt_pool.tile([P, NT, 129], f32, tag="dbg_pv")
        for mt in range(NT):
            c0 = pv_col(mt)
            nc.vector.tensor_copy(pvc[:, mt, :], pv[:, c0:c0 + 129])
        nc.sync.dma_start(dbg["pv"], pvc[:])

    # --- finalize: divide by denominator, store ---------------------------
    fin_pool = ctx.enter_context(tc.tile_pool(name="fin", bufs=2))
    for mt in range(NT):
        c0 = pv_col(mt)
        rec = fin_pool.tile([P, 1], f32, tag="rec")
        nc.vector.reciprocal(rec[:], pv[:, c0 + 128:c0 + 129])
        osb = fin_pool.tile([P, OUT_DIM], f32, tag="osb")
        nc.vector.tensor_scalar(
            osb[:], pv[:, c0:c0 + 128], rec[:], None, mybir.AluOpType.mult)
        nc.sync.dma_start(out_r[:, mt, :], osb[:])


def build_kernel(debug=False):
    nc = bacc.Bacc("TRN2", target_bir_lowering=False, debug=False,
                   num_devices=NCORES)
    x_d = nc.dram_tensor("x", (E, IN_DIM), f32, kind="ExternalInput")
    xl_d = nc.dram_tensor("x_local", (R, IN_DIM), f32, kind="ExternalInput")
    wq_d = nc.dram_tensor("wq", (IN_DIM, OUT_DIM), f32, kind="ExternalInput")
    wk_d = nc.dram_tensor("wk", (IN_DIM, OUT_DIM), f32, kind="ExternalInput")
    mask_d = nc.dram_tensor("mask", (R, E), i32, kind="ExternalInput")
    out_d = nc.dram_tensor("out", (R, OUT_DIM), f32, kind="ExternalOutput")

    dbg = None
    if debug:
        dbg = {
            "qT": nc.dram_tensor("dbg_qT", (P, R), f32, kind="ExternalOutput").ap(),
            "kT": nc.dram_tensor("dbg_kT", (P, E), f32, kind="ExternalOutput").ap(),
            "kaug": nc.dram_tensor("dbg_kaug", (P, NJ, 129), bf16,
                                   kind="ExternalOutput").ap(),
            "pt0": nc.dram_tensor("dbg_pt0", (P, R), bf16,
                                  kind="ExternalOutput").ap(),
            "st0": nc.dram_tensor("dbg_st0", (P, R), f32,
                                  kind="ExternalOutput").ap(),
            "pv": nc.dram_tensor("dbg_pv", (P, NT, 129), f32,
                                 kind="ExternalOutput").ap(),
        }

    with tile.TileContext(nc) as tc:
        with ExitStack() as ctx:
            _body(ctx, tc, out_d.ap(), x_d.ap(), xl_d.ap(), wq_d.ap(),
                  wk_d.ap(), mask_d.ap(), dbg=dbg)
    nc.compile()
    return nc


_CACHED = {}


def _get_nc():
    if "nc" not in _CACHED:
        _CACHED["nc"] = build_kernel()
    return _CACHED["nc"]


def kernel(node_embeddings, query_weight, key_weight, attention_mask,
           _trace=False):
    x = np.ascontiguousarray(np.asarray(node_embeddings, dtype=np.float32))
    wq = np.ascontiguousarray(np.asarray(query_weight, dtype=np.float32))
    wk = np.ascontiguousarray(np.asarray(key_weight, dtype=np.float32))
    mask = np.ascontiguousarray(np.asarray(attention_mask, dtype=np.int32))

    nc = _get_nc()
    in_maps = []
    for c in range(NCORES):
        sl = slice(c * R, (c + 1) * R)
        in_maps.append({
            "x": x,
            "x_local": x[sl],
            "wq": wq,
            "wk": wk,
            "mask": mask[sl],
        })
    res = bass_utils.run_bass_kernel_spmd(
        nc, in_maps, core_ids=list(range(NCORES)), trace=_trace)
    out = np.concatenate([res.results[c]["out"] for c in range(NCORES)], axis=0)
    if _trace:
        _CACHED["last_results"] = res
    return out


if __name__ == "__main__":
    # quick smoke test with random data
    rng = np.random.default_rng(0)
    x = rng.standard_normal((E, IN_DIM), dtype=np.float32)
    wq = rng.standard_normal((IN_DIM, OUT_DIM), dtype=np.float32) * 0.1
    wk = rng.standard_normal((IN_DIM, OUT_DIM), dtype=np.float32) * 0.1
    mask = rng.integers(0, 2, size=(E, E)).astype(np.int32)
    np.fill_diagonal(mask, 1)
    out = kernel(x, wq, wk, mask)
    print("out", out.shape, out.dtype)


# revision 21
# speedup vs baseline: 1.6043x; 1.6043x over previous
"""Trainium2 Bass kernel for NodeAttAggregator (masked self-attention aggregation).

Math (per reference):
    q = x @ Wq ; k = x @ Wk                       [E,128]
    s = (q @ k.T) / sqrt(128)                     [E,E]
    p = softmax(s masked where mask==0 -> -inf)   row-wise
    out = p @ k                                   [E,128]

Sharding: row-parallel over 8 cores. Core c owns query rows [c*1024,(c+1)*1024).
Each core receives the full x (for k), its local x slice (for q), and its local
mask rows; it computes its 1024 output rows. Host concatenates.

On-core algorithm (per core, R=1024 local rows):
  setup: cast x to fp16, transpose on PE (identity matmuls) -> xT; k = x@Wk
         (fp16 in, f32 psum); kT (fp16, for scores), k_aug (bf16 + ones col,
         for PV); qT = Wq.T@xT_local (fp16).
  main loop over 64 key chunks j (128 keys each):
     ST_psum[kpos,qpos] = matmul(lhsT=kT_chunk, rhs=qT)  (scores TRANSPOSED so
         the exp output is directly P.T, which the PV matmul needs as weights)
       + CMASK*mask.T via matmul(lhsT=mask_tile_bf16, rhs=CMASK*I) which
         transposes the mask on the PE for free while accumulating into PSUM
     PT = exp(SCALE*ST + BIAS) on ACT (PSUM->SBUF, bf16)  [softmax w/o rowmax:
         constant shift C=8; masked lanes underflow to ~0]
     PV: out_psum[m,129] += PT_chunk.T @ k_aug_chunk  (ones col = denominator)
  final: out = out_psum[:, :128] * (1/out_psum[:,128]) ; DMA out.

Unnormalized softmax: out_row = sum_j p_j k_j / sum_j p_j; constant shift is
safe (scores/sqrt(d) in [-19.6, 18.5] for this data; all p fit bf16 range).
PSUM note: matmul start=True clears the whole BANK, so only the first writer
of each bank sets it.
"""

import sys

for _p in ("/opt/trn_rl_repo",):
    if _p not in sys.path:
        sys.path.insert(0, _p)

from contextlib import ExitStack

import numpy as np

import concourse.bacc as bacc
import concourse.bass as bass
import concourse.tile as tile
from concourse import masks, mybir
from concourse import bass_utils

E = 8192
IN_DIM = 256
OUT_DIM = 128
NCORES = 8
R = E // NCORES            # 1024 local query rows per core
P = 128                    # partitions
NJ = E // P                # 64 key chunks
NT = R // P                # 8 local row tiles
DH = IN_DIM // P           # 2 halves of the contraction dim
SCALE = 1.0 / float(np.sqrt(np.float32(OUT_DIM)))
CMASK = 1024.0             # additive mask constant (applied via identity matmul)
CSHIFT = 8.0               # constant softmax shift (instead of row max)
EXP_BIAS = -CMASK * SCALE - CSHIFT

f32 = mybir.dt.float32
bf16 = mybir.dt.bfloat16
f16 = mybir.dt.float16
i32 = mybir.dt.int32


def _body(ctx: ExitStack, tc: tile.TileContext, out_d, x_d, xl_d, wq_d, wk_d, mask_d,
          dbg=None):
    nc = tc.nc

    # DRAM views
    x_r = x_d.rearrange("(c p) d -> p c d", p=P)          # [128, 64, 256]
    xl_r = xl_d.rearrange("(c p) d -> p c d", p=P)        # [128, 8, 256]
    wq_r = wq_d.rearrange("(h p) d -> p h d", p=P)        # [128, 2, 128]
    wk_r = wk_d.rearrange("(h p) d -> p h d", p=P)
    # blocks of 4 key-chunks: per (partition,t) row a contiguous 2 KB segment
    mask_r = mask_d.rearrange("(t p) (jb f) -> p t jb f", p=P, f=4 * P)  # [128,8,16,512]
    out_r = out_d.rearrange("(t p) d -> p t d", p=P)      # [128, 8, 128]

    # --- persistent tiles -------------------------------------------------
    persist = ctx.enter_context(tc.tile_pool(name="persist", bufs=1))
    ident_h = persist.tile([P, P], f16, tag="ident_h")      # I (fp16, transposes)
    ident_c = persist.tile([P, P], bf16, tag="ident_c")     # CMASK * I
    bias_sb = persist.tile([P, 1], f32, tag="bias")
    kT = persist.tile([P, E], f16, tag="kT")                # k transposed [d, kpos]
    qT = persist.tile([P, R], f16, tag="qT")                # q transposed [d, m]
    k_aug = persist.tile([P, NJ, 132], bf16, tag="k_aug")   # k chunks + ones col
    wq_sb = persist.tile([P, DH, P], f16, tag="wq")
    wk_sb = persist.tile([P, DH, P], f16, tag="wk")

    nc.gpsimd.memset(bias_sb[:], EXP_BIAS)
    masks.make_identity(nc, ident_h[:])
    nc.gpsimd.memset(ident_c[:], 0.0)
    nc.gpsimd.affine_select(
        out=ident_c[:], in_=ident_c[:],
        compare_op=mybir.AluOpType.not_equal,
        fill=CMASK, base=0, pattern=[[-1, P]], channel_multiplier=1,
    )
    nc.gpsimd.memset(k_aug[:, :, 128:129], 1.0)

    # --- setup phase: xT, k, kT, qT (fp16 PE path) ------------------------
    with tc.tile_pool(name="setup_sb", bufs=2) as ssb, \
         tc.tile_pool(name="setup_xT", bufs=1) as sxT, \
         tc.tile_pool(name="ps_a", bufs=3, space="PSUM") as psa, \
         tc.tile_pool(name="ps_b", bufs=2, space="PSUM") as psb:

        w_f32 = ssb.tile([P, DH, 2 * P], f32, tag="w_f32")
        nc.sync.dma_start(w_f32[:, :, 0:P], wq_r)
        nc.sync.dma_start(w_f32[:, :, P:2 * P], wk_r)
        nc.any.tensor_copy(wq_sb[:], w_f32[:, :, 0:P])
        nc.any.tensor_copy(wk_sb[:], w_f32[:, :, P:2 * P])

        xT = sxT.tile([P, DH, E], f16, tag="xT")        # x transposed (full)
        xTl = sxT.tile([P, DH, R], f16, tag="xTl")      # x_local transposed

        # cast x to fp16 then transpose: 8 transposes packed per PSUM bank
        def transpose_block(xh_view, dst_view, n_rc):
            # xh_view: [P, n_rc, IN_DIM] fp16; dst: [P, DH, n_rc*P] slices
            for h in range(DH):
                pt = psa.tile([P, 8 * P], f16, tag="tr")
                for rc in range(n_rc):
                    nc.tensor.matmul(
                        pt[:, rc * P:(rc + 1) * P],
                        xh_view[:, rc, h * P:(h + 1) * P], ident_h[:],
                        is_transpose=True,
                        start=(rc == 0), stop=(rc == n_rc - 1),
                        skip_group_check=True)
                nc.any.tensor_copy(dst_view[:, h, 0:n_rc * P], pt[:, 0:n_rc * P])

        for cc in range(8):
            x_sb = ssb.tile([P, 8, IN_DIM], f32, tag="x_sb")
            nc.sync.dma_start(x_sb[:], x_r[:, cc * 8:(cc + 1) * 8, :])
            x_h = ssb.tile([P, 8, IN_DIM], f16, tag="x_h")
            nc.vector.tensor_copy(x_h[:], x_sb[:])
            transpose_block(x_h, xT[:, :, cc * 8 * P:(cc + 1) * 8 * P], 8)
        # x_local
        xl_sb = ssb.tile([P, 8, IN_DIM], f32, tag="x_sb")
        nc.sync.dma_start(xl_sb[:], xl_r[:])
        xl_h = ssb.tile([P, 8, IN_DIM], f16, tag="x_h")
        nc.vector.tensor_copy(xl_h[:], xl_sb[:])
        transpose_block(xl_h, xTl, 8)

        # k (natural) -> k_aug bf16: k_chunk = x_chunk @ Wk
        for col in range(NJ):
            pk = psa.tile([P, P], f32, tag="kproj")
            for h in range(DH):
                nc.tensor.matmul(
                    pk[:], xT[:, h, col * P:(col + 1) * P], wk_sb[:, h, :],
                    start=(h == 0), stop=(h == DH - 1))
            nc.any.tensor_copy(k_aug[:, col, 0:128], pk[:])

        # kT = Wk.T @ xT   [d, kpos] fp16
        for n in range(E // 512):
            pkt = psb.tile([P, 512], f32, tag="ktproj")
            for h in range(DH):
                nc.tensor.matmul(
                    pkt[:], wk_sb[:, h, :], xT[:, h, n * 512:(n + 1) * 512],
                    start=(h == 0), stop=(h == DH - 1))
            nc.any.tensor_copy(kT[:, n * 512:(n + 1) * 512], pkt[:])

        # qT = Wq.T @ xT_local   [d, m] fp16
        for n in range(R // 512):
            pqt = psb.tile([P, 512], f32, tag="ktproj")
            for h in range(DH):
                nc.tensor.matmul(
                    pqt[:], wq_sb[:, h, :], xTl[:, h, n * 512:(n + 1) * 512],
                    start=(h == 0), stop=(h == DH - 1))
            nc.any.tensor_copy(qT[:, n * 512:(n + 1) * 512], pqt[:])

    # --- main loop --------------------------------------------------------
    mi_pool = ctx.enter_context(tc.tile_pool(name="mask_i32", bufs=3))
    mb_pool = ctx.enter_context(tc.tile_pool(name="mask_bf16", bufs=2))
    pt_pool = ctx.enter_context(tc.tile_pool(name="pt", bufs=3))
    st_pool = ctx.enter_context(tc.tile_pool(name="st_psum", bufs=2, space="PSUM"))
    pv_pool = ctx.enter_context(tc.tile_pool(name="pv_psum", bufs=1, space="PSUM"))

    pv = pv_pool.tile([P, 1536], f32)   # 3 banks; 8 slots of 129 (+ ones col)

    def pv_col(mt):
        g, s = divmod(mt, 3)
        return g * 512 + s * 129

    for j in range(NJ):
        if j % 4 == 0:
            mi = mi_pool.tile([P, NT, 4 * P], i32)
            nc.sync.dma_start(mi[:], mask_r[:, :, j // 4, :])
            mb = mb_pool.tile([P, NT, 4 * P], bf16)
            nc.vector.tensor_copy(mb[:], mi[:])
        js = j % 4

        st = st_pool.tile([P, R], f32)   # [kpos 128, qpos 1024] = 2 banks
        # scores (transposed): st = kT_chunk-as-weights @ qT   (fp16)
        for n in range(R // 512):
            nc.tensor.matmul(
                st[:, n * 512:(n + 1) * 512],
                kT[:, j * P:(j + 1) * P], qT[:, n * 512:(n + 1) * 512],
                start=True, stop=False, skip_group_check=True)
        # + CMASK * mask.T  (PE transposes the mask while accumulating)
        for t in range(NT):
            nc.tensor.matmul(
                st[:, t * P:(t + 1) * P],
                mb[:, t, js * P:(js + 1) * P], ident_c[:],
                start=False, stop=True, skip_group_check=True)

        ptile = pt_pool.tile([P, R], bf16)
        nc.scalar.activation(
            ptile[:], st[:], mybir.ActivationFunctionType.Exp,
            bias=bias_sb[:], scale=SCALE)
        if dbg is not None and j == 0:
            nc.sync.dma_start(dbg["pt0"], ptile[:])
            stc = pt_pool.tile([P, R], f32, tag="dbg_st")
            nc.vector.tensor_copy(stc[:], st[:])
            nc.sync.dma_start(dbg["st0"], stc[:])

        # start=True clears the whole PSUM bank -> only the FIRST slot of
        # each bank may set it (at j==0), or later slots wipe earlier ones.
        for mt in range(NT):
            g, s = divmod(mt, 3)
            last_in_bank = (mt == NT - 1) or (s == 2)
            c0 = pv_col(mt)
            nc.tensor.matmul(
                pv[:, c0:c0 + 129],
                ptile[:, mt * P:(mt + 1) * P], k_aug[:, j, 0:129],
                start=(j == 0 and s == 0),
                stop=(j == NJ - 1 and last_in_bank),
                skip_group_check=True)

    if dbg is not None:
        nc.sync.dma_start(dbg["qT"], qT[:])
        nc.sync.dma_start(dbg["kT"], kT[:])
        nc.sync.dma_start(dbg["kaug"], k_aug[:, :, 0:129])
        pvc = pt_pool.tile([P, NT, 129], f32, tag="dbg_pv")
        for mt in range(NT):
            c0 = pv_col(mt)
            nc.vector.tensor_copy(pvc[:, mt, :], pv[:, c0:c0 + 129])
        nc.sync.dma_start(dbg["pv"], pvc[:])

    # --- finalize: divide by denominator, store ---------------------------
    fin_pool = ctx.enter_context(tc.tile_pool(name="fin", bufs=2))
    for mt in range(NT):
        c0 = pv_col(mt)
        rec = fin_pool.tile([P, 1], f32, tag="rec")
        nc.vector.reciprocal(rec[:], pv[:, c0 + 128:c0 + 129])
        osb = fin_pool.tile([P, OUT_DIM], f32, tag="osb")
        nc.vector.tensor_scalar(
            osb[:], pv[:, c0:c0 + 128], rec[:], None, mybir.AluOpType.mult)
        nc.sync.dma_start(out_r[:, mt, :], osb[:])


def build_kernel(debug=False):
    nc = bacc.Bacc("TRN2", target_bir_lowering=False, debug=False,
                   num_devices=NCORES)
    x_d = nc.dram_tensor("x", (E, IN_DIM), f32, kind="ExternalInput")
    xl_d = nc.dram_tensor("x_local", (R, IN_DIM), f32, kind="ExternalInput")
    wq_d = nc.dram_tensor("wq", (IN_DIM, OUT_DIM), f32, kind="ExternalInput")
    wk_d = nc.dram_tensor("wk", (IN_DIM, OUT_DIM), f32, kind="ExternalInput")
    mask_d = nc.dram_tensor("mask", (R, E), i32, kind="ExternalInput")
    out_d = nc.dram_tensor("out", (R, OUT_DIM), f32, kind="ExternalOutput")

    dbg = None
    if debug:
        dbg = {
            "qT": nc.dram_tensor("dbg_qT", (P, R), f16, kind="ExternalOutput").ap(),
            "kT": nc.dram_tensor("dbg_kT", (P, E), f16, kind="ExternalOutput").ap(),
            "kaug": nc.dram_tensor("dbg_kaug", (P, NJ, 129), bf16,
                                   kind="ExternalOutput").ap(),
            "pt0": nc.dram_tensor("dbg_pt0", (P, R), bf16,
                                  kind="ExternalOutput").ap(),
            "st0": nc.dram_tensor("dbg_st0", (P, R), f32,
                                  kind="ExternalOutput").ap(),
            "pv": nc.dram_tensor("dbg_pv", (P, NT, 129), f32,
                                 kind="ExternalOutput").ap(),
        }

    with tile.TileContext(nc) as tc:
        with ExitStack() as ctx:
            _body(ctx, tc, out_d.ap(), x_d.ap(), xl_d.ap(), wq_d.ap(),
                  wk_d.ap(), mask_d.ap(), dbg=dbg)
    nc.compile()
    return nc


_CACHED = {}


def _get_nc():
    if "nc" not in _CACHED:
        _CACHED["nc"] = build_kernel()
    return _CACHED["nc"]


def kernel(node_embeddings, query_weight, key_weight, attention_mask,
           _trace=False):
    x = np.ascontiguousarray(np.asarray(node_embeddings, dtype=np.float32))
    wq = np.ascontiguousarray(np.asarray(query_weight, dtype=np.float32))
    wk = np.ascontiguousarray(np.asarray(key_weight, dtype=np.float32))
    mask = np.ascontiguousarray(np.asarray(attention_mask, dtype=np.int32))

    nc = _get_nc()
    in_maps = []
    for c in range(NCORES):
        sl = slice(c * R, (c + 1) * R)
        in_maps.append({
            "x": x,
            "x_local": x[sl],
            "wq": wq,
            "wk": wk,
            "mask": mask[sl],
        })
    res = bass_utils.run_bass_kernel_spmd(
        nc, in_maps, core_ids=list(range(NCORES)), trace=_trace)
    out = np.concatenate([res.results[c]["out"] for c in range(NCORES)], axis=0)
    if _trace:
        _CACHED["last_results"] = res
    return out


if __name__ == "__main__":
    rng = np.random.default_rng(0)
    x = rng.standard_normal((E, IN_DIM), dtype=np.float32)
    wq = rng.standard_normal((IN_DIM, OUT_DIM), dtype=np.float32) * 0.1
    wk = rng.standard_normal((IN_DIM, OUT_DIM), dtype=np.float32) * 0.1
    mask = rng.integers(0, 2, size=(E, E)).astype(np.int32)
    np.fill_diagonal(mask, 1)
    out = kernel(x, wq, wk, mask)
    print("out", out.shape, out.dtype)


# revision 23
# speedup vs baseline: 1.6254x; 1.0132x over previous
"""Trainium2 Bass kernel for NodeAttAggregator (masked self-attention aggregation).

Math (per reference):
    q = x @ Wq ; k = x @ Wk                       [E,128]
    s = (q @ k.T) / sqrt(128)                     [E,E]
    p = softmax(s masked where mask==0 -> -inf)   row-wise
    out = p @ k                                   [E,128]

Sharding: row-parallel over 8 cores. Core c owns query rows [c*1024,(c+1)*1024).
Each core receives the full x (for k), its local x slice (for q), and its local
mask rows; it computes its 1024 output rows. Host concatenates.

On-core algorithm (per core, R=1024 local rows), fp16 scores / bf16 PV:
  prologue: x_local -> fp16 -> PE-transpose -> qT = Wq.T @ xT_local (fp16).
  fused loop over 8 column blocks cc (1024 keys each):
     x block -> fp16 -> PE-transpose -> xT_blk;
     k_aug[cc] = x_blk @ Wk (bf16 + ones col);  kT[cc] = Wk.T @ xT_blk (fp16)
     mask block DMA (4 MB, int32) -> bf16
     for 8 key chunks j in the block:
        ST_psum[kpos,qpos] = matmul(lhsT=kT_chunk, rhs=qT)   (scores TRANSPOSED
            so exp output is directly P.T, which the PV matmul needs as weights)
          + CMASK*mask.T via matmul(lhsT=mask_tile, rhs=CMASK*I): the PE
            transposes the mask for free while accumulating into PSUM
        PT = exp(SCALE*ST + BIAS) on ACT (PSUM->SBUF, bf16)  [softmax w/o row
            max: constant shift C=8; masked lanes underflow to ~0]
        PV: out_psum[m,129] += PT_chunk.T @ k_aug_chunk (ones col = denominator)
  epilogue: out = out_psum[:, :128] * (1/out_psum[:,128]) ; DMA out.

Unnormalized softmax: out_row = sum_j p_j k_j / sum_j p_j; constant shift is
safe (scores/sqrt(d) in [-19.6, 18.5] for this data; all p fit bf16 range).
PSUM note: matmul start=True clears the whole BANK, so only the first writer
of each bank sets it.
"""

import sys

for _p in ("/opt/trn_rl_repo",):
    if _p not in sys.path:
        sys.path.insert(0, _p)

from contextlib import ExitStack

import numpy as np

import concourse.bacc as bacc
import concourse.bass as bass
import concourse.tile as tile
from concourse import masks, mybir
from concourse import bass_utils

E = 8192
IN_DIM = 256
OUT_DIM = 128
NCORES = 8
R = E // NCORES            # 1024 local query rows per core
P = 128                    # partitions
NJ = E // P                # 64 key chunks
NT = R // P                # 8 local row tiles
NB = 8                     # column blocks (8 key chunks each)
DH = IN_DIM // P           # 2 halves of the contraction dim
SCALE = 1.0 / float(np.sqrt(np.float32(OUT_DIM)))
CMASK = 1024.0             # additive mask constant (applied via identity matmul)
CSHIFT = 8.0               # constant softmax shift (instead of row max)
EXP_BIAS = -CMASK * SCALE - CSHIFT

f32 = mybir.dt.float32
bf16 = mybir.dt.bfloat16
f16 = mybir.dt.float16
i32 = mybir.dt.int32


def _body(ctx: ExitStack, tc: tile.TileContext, out_d, x_d, xl_d, wq_d, wk_d, mask_d,
          dbg=None):
    nc = tc.nc

    # DRAM views
    x_r = x_d.rearrange("(c p) d -> p c d", p=P)          # [128, 64, 256]
    xl_r = xl_d.rearrange("(c p) d -> p c d", p=P)        # [128, 8, 256]
    wq_r = wq_d.rearrange("(h p) d -> p h d", p=P)        # [128, 2, 128]
    wk_r = wk_d.rearrange("(h p) d -> p h d", p=P)
    # one block = 8 key chunks: per (partition,t) row a contiguous 4 KB segment
    mask_r = mask_d.rearrange("(t p) (b f) -> p t b f", p=P, f=NJ // NB * P)
    out_r = out_d.rearrange("(t p) d -> p t d", p=P)      # [128, 8, 128]

    # --- persistent tiles -------------------------------------------------
    persist = ctx.enter_context(tc.tile_pool(name="persist", bufs=1))
    ident_h = persist.tile([P, P], f16, tag="ident_h")      # I (fp16, transposes)
    ident_c = persist.tile([P, P], bf16, tag="ident_c")     # CMASK * I
    bias_sb = persist.tile([P, 1], f32, tag="bias")
    kT = persist.tile([P, E], f16, tag="kT")                # k transposed [d, kpos]
    qT = persist.tile([P, R], f16, tag="qT")                # q transposed [d, m]
    k_aug = persist.tile([P, NJ, 132], bf16, tag="k_aug")   # k chunks + ones col
    wq_sb = persist.tile([P, DH, P], f16, tag="wq")
    wk_sb = persist.tile([P, DH, P], f16, tag="wk")

    nc.gpsimd.memset(bias_sb[:], EXP_BIAS)
    masks.make_identity(nc, ident_h[:])
    nc.gpsimd.memset(ident_c[:], 0.0)
    nc.gpsimd.affine_select(
        out=ident_c[:], in_=ident_c[:],
        compare_op=mybir.AluOpType.not_equal,
        fill=CMASK, base=0, pattern=[[-1, P]], channel_multiplier=1,
    )
    nc.gpsimd.memset(k_aug[:, :, 128:129], 1.0)

    # pools
    xsb_pool = ctx.enter_context(tc.tile_pool(name="x_sb", bufs=2))
    xh_pool = ctx.enter_context(tc.tile_pool(name="x_h", bufs=2))
    xT_pool = ctx.enter_context(tc.tile_pool(name="xT_blk", bufs=2))
    mi_pool = ctx.enter_context(tc.tile_pool(name="mask_i32", bufs=2))
    mb_pool = ctx.enter_context(tc.tile_pool(name="mask_bf16", bufs=2))
    pt_pool = ctx.enter_context(tc.tile_pool(name="pt", bufs=4))
    ps_tr = ctx.enter_context(tc.tile_pool(name="ps_tr", bufs=2, space="PSUM"))
    st_pool = ctx.enter_context(tc.tile_pool(name="st_psum", bufs=3, space="PSUM"))
    pv_pool = ctx.enter_context(tc.tile_pool(name="pv_psum", bufs=1, space="PSUM"))

    pv = pv_pool.tile([P, 1536], f32)   # 3 banks; 8 slots of 129 (+ ones col)

    def pv_col(mt):
        g, s = divmod(mt, 3)
        return g * 512 + s * 129

    def transpose_block(xh_view, dst_view):
        # xh_view: [P, 8, IN_DIM] fp16; dst_view: [P, DH, 8*P] fp16
        for h in range(DH):
            pt = ps_tr.tile([P, 8 * P], f16, tag="w")
            for rc in range(8):
                nc.tensor.matmul(
                    pt[:, rc * P:(rc + 1) * P],
                    xh_view[:, rc, h * P:(h + 1) * P], ident_h[:],
                    is_transpose=True,
                    start=(rc == 0), stop=(rc == 7),
                    skip_group_check=True)
            nc.vector.tensor_copy(dst_view[:, h, :], pt[:])

    # --- prologue: weights + qT ------------------------------------------
    w_f32 = xsb_pool.tile([P, DH, 2 * P], f32, tag="w_f32")
    nc.sync.dma_start(w_f32[:, :, 0:P], wq_r)
    nc.sync.dma_start(w_f32[:, :, P:2 * P], wk_r)
    nc.vector.tensor_copy(wq_sb[:], w_f32[:, :, 0:P])
    nc.vector.tensor_copy(wk_sb[:], w_f32[:, :, P:2 * P])

    xl_sb = xsb_pool.tile([P, 8, IN_DIM], f32, tag="x_sb")
    nc.sync.dma_start(xl_sb[:], xl_r[:])
    xl_h = xh_pool.tile([P, 8, IN_DIM], f16, tag="x_h")
    nc.vector.tensor_copy(xl_h[:], xl_sb[:])
    xTl = xT_pool.tile([P, DH, R], f16, tag="xTl")
    transpose_block(xl_h, xTl)
    for n in range(R // 512):
        pqt = ps_tr.tile([P, 512], f32, tag="w")
        for h in range(DH):
            nc.tensor.matmul(
                pqt[:], wq_sb[:, h, :], xTl[:, h, n * 512:(n + 1) * 512],
                start=(h == 0), stop=(h == DH - 1))
        nc.vector.tensor_copy(qT[:, n * 512:(n + 1) * 512], pqt[:])

    # --- fused main loop --------------------------------------------------
    for cc in range(NB):
        # x block -> fp16 -> transposed
        x_sb = xsb_pool.tile([P, 8, IN_DIM], f32, tag="x_sb")
        nc.sync.dma_start(x_sb[:], x_r[:, cc * 8:(cc + 1) * 8, :])
        x_h = xh_pool.tile([P, 8, IN_DIM], f16, tag="x_h")
        nc.vector.tensor_copy(x_h[:], x_sb[:])
        xT_blk = xT_pool.tile([P, DH, 8 * P], f16, tag="xT_blk")
        transpose_block(x_h, xT_blk)

        # k_aug chunks for this block (bf16) + kT columns (fp16)
        for rc in range(8):
            pk = ps_tr.tile([P, P], f32, tag="w")
            for h in range(DH):
                nc.tensor.matmul(
                    pk[:], xT_blk[:, h, rc * P:(rc + 1) * P], wk_sb[:, h, :],
                    start=(h == 0), stop=(h == DH - 1))
            nc.vector.tensor_copy(k_aug[:, cc * 8 + rc, 0:128], pk[:])
        for n in range(2):
            pkt = ps_tr.tile([P, 512], f32, tag="w")
            for h in range(DH):
                nc.tensor.matmul(
                    pkt[:], wk_sb[:, h, :],
                    xT_blk[:, h, n * 512:(n + 1) * 512],
                    start=(h == 0), stop=(h == DH - 1))
            nc.vector.tensor_copy(
                kT[:, cc * 1024 + n * 512:cc * 1024 + (n + 1) * 512], pkt[:])

        # mask block (8 chunks, 4 MB) -> bf16
        mi = mi_pool.tile([P, NT, 8 * P], i32)
        nc.sync.dma_start(mi[:], mask_r[:, :, cc, :])
        mb = mb_pool.tile([P, NT, 8 * P], bf16)
        nc.vector.tensor_copy(mb[:], mi[:])

        for js in range(8):
            j = cc * 8 + js
            ptile = pt_pool.tile([P, R], bf16)
            for n in range(2):   # qpos halves of 512
                st = st_pool.tile([P, 512], f32)   # 1 bank
                nc.tensor.matmul(
                    st[:], kT[:, j * P:(j + 1) * P],
                    qT[:, n * 512:(n + 1) * 512],
                    start=True, stop=False, skip_group_check=True)
                for t4 in range(4):
                    t = n * 4 + t4
                    nc.tensor.matmul(
                        st[:, t4 * P:(t4 + 1) * P],
                        mb[:, t, js * P:(js + 1) * P], ident_c[:],
                        start=False, stop=True, skip_group_check=True)
                nc.scalar.activation(
                    ptile[:, n * 512:(n + 1) * 512], st[:],
                    mybir.ActivationFunctionType.Exp,
                    bias=bias_sb[:], scale=SCALE)

            if dbg is not None and j == 0:
                nc.sync.dma_start(dbg["pt0"], ptile[:])

            # start=True clears the whole PSUM bank -> only the FIRST slot of
            # each bank may set it (at j==0).
            for mt in range(NT):
                g, s = divmod(mt, 3)
                last_in_bank = (mt == NT - 1) or (s == 2)
                c0 = pv_col(mt)
                nc.tensor.matmul(
                    pv[:, c0:c0 + 129],
                    ptile[:, mt * P:(mt + 1) * P], k_aug[:, j, 0:129],
                    start=(j == 0 and s == 0),
                    stop=(j == NJ - 1 and last_in_bank),
                    skip_group_check=True)

    if dbg is not None:
        nc.sync.dma_start(dbg["qT"], qT[:])
        nc.sync.dma_start(dbg["kT"], kT[:])
        nc.sync.dma_start(dbg["kaug"], k_aug[:, :, 0:129])
        pvc = persist.tile([P, NT, 129], f32, tag="dbg_pv")
        for mt in range(NT):
            c0 = pv_col(mt)
            nc.vector.tensor_copy(pvc[:, mt, :], pv[:, c0:c0 + 129])
        nc.sync.dma_start(dbg["pv"], pvc[:])

    # --- finalize: divide by denominator, store ---------------------------
    fin_pool = ctx.enter_context(tc.tile_pool(name="fin", bufs=2))
    for mt in range(NT):
        c0 = pv_col(mt)
        rec = fin_pool.tile([P, 1], f32, tag="rec")
        nc.vector.reciprocal(rec[:], pv[:, c0 + 128:c0 + 129])
        osb = fin_pool.tile([P, OUT_DIM], f32, tag="osb")
        nc.vector.tensor_scalar(
            osb[:], pv[:, c0:c0 + 128], rec[:], None, mybir.AluOpType.mult)
        nc.sync.dma_start(out_r[:, mt, :], osb[:])


def build_kernel(debug=False):
    nc = bacc.Bacc("TRN2", target_bir_lowering=False, debug=False,
                   num_devices=NCORES)
    x_d = nc.dram_tensor("x", (E, IN_DIM), f32, kind="ExternalInput")
    xl_d = nc.dram_tensor("x_local", (R, IN_DIM), f32, kind="ExternalInput")
    wq_d = nc.dram_tensor("wq", (IN_DIM, OUT_DIM), f32, kind="ExternalInput")
    wk_d = nc.dram_tensor("wk", (IN_DIM, OUT_DIM), f32, kind="ExternalInput")
    mask_d = nc.dram_tensor("mask", (R, E), i32, kind="ExternalInput")
    out_d = nc.dram_tensor("out", (R, OUT_DIM), f32, kind="ExternalOutput")

    dbg = None
    if debug:
        dbg = {
            "qT": nc.dram_tensor("dbg_qT", (P, R), f16, kind="ExternalOutput").ap(),
            "kT": nc.dram_tensor("dbg_kT", (P, E), f16, kind="ExternalOutput").ap(),
            "kaug": nc.dram_tensor("dbg_kaug", (P, NJ, 129), bf16,
                                   kind="ExternalOutput").ap(),
            "pt0": nc.dram_tensor("dbg_pt0", (P, R), bf16,
                                  kind="ExternalOutput").ap(),
            "pv": nc.dram_tensor("dbg_pv", (P, NT, 129), f32,
                                 kind="ExternalOutput").ap(),
        }

    with tile.TileContext(nc) as tc:
        with ExitStack() as ctx:
            _body(ctx, tc, out_d.ap(), x_d.ap(), xl_d.ap(), wq_d.ap(),
                  wk_d.ap(), mask_d.ap(), dbg=dbg)
    nc.compile()
    return nc


_CACHED = {}


def _get_nc():
    if "nc" not in _CACHED:
        _CACHED["nc"] = build_kernel()
    return _CACHED["nc"]


def kernel(node_embeddings, query_weight, key_weight, attention_mask,
           _trace=False):
    x = np.ascontiguousarray(np.asarray(node_embeddings, dtype=np.float32))
    wq = np.ascontiguousarray(np.asarray(query_weight, dtype=np.float32))
    wk = np.ascontiguousarray(np.asarray(key_weight, dtype=np.float32))
    mask = np.ascontiguousarray(np.asarray(attention_mask, dtype=np.int32))

    nc = _get_nc()
    in_maps = []
    for c in range(NCORES):
        sl = slice(c * R, (c + 1) * R)
        in_maps.append({
            "x": x,
            "x_local": x[sl],
            "wq": wq,
            "wk": wk,
            "mask": mask[sl],
        })
    res = bass_utils.run_bass_kernel_spmd(
        nc, in_maps, core_ids=list(range(NCORES)), trace=_trace)
    out = np.concatenate([res.results[c]["out"] for c in range(NCORES)], axis=0)
    if _trace:
        _CACHED["last_results"] = res
    return out


if __name__ == "__main__":
    rng = np.random.default_rng(0)
    x = rng.standard_normal((E, IN_DIM), dtype=np.float32)
    wq = rng.standard_normal((IN_DIM, OUT_DIM), dtype=np.float32) * 0.1
    wk = rng.standard_normal((IN_DIM, OUT_DIM), dtype=np.float32) * 0.1
    mask = rng.integers(0, 2, size=(E, E)).astype(np.int32)
    np.fill_diagonal(mask, 1)
    out = kernel(x, wq, wk, mask)
    print("out", out.shape, out.dtype)
